# revision 5
# baseline (speedup 1.0000x reference)
"""Trainium2 Bass kernel for nn_AttentionBlock (B=1, C=512, T=8, H=W=64).

Math: the reference's attention has seq-len 1 (softmax over a single
element == 1.0), so o == v and Q/K never affect the output:

    out = x + s(px) * (W_eff @ x)(px) + b_eff
    W_eff = w_proj @ w_v * gamma,  w_v = w_qkv[2C:3C]
    b_eff = w_proj @ b_v + b_proj
    s(px) = sqrt(C) / clip(||x[:, px]||, 1e-12)

(The per-pixel RMS scale s commutes through the channel contraction, so
the GEMM runs on raw x and s is applied to the GEMM output.)

Sharding: data-parallel over the fused (b*t)=8 frame axis, one frame per
NeuronCore; weights replicated. Per core the frame is shipped tile-major
([tile, p, chunk, n]: channels on partitions, pixels on the free dim,
one contiguous 1MB DRAM block per 512-pixel tile).

Engine budget per 512-pixel tile (8 tiles/core), measured against the
79.7us baseline trace whose Vector engine (47us busy) gated the store
stream:
  PE      16 mains + 2 sumsq-ones MMs            ~4.2us
  ACT     square + fused rsqrt(sumsq/C+eps)      ~2.5us
  GPSIMD  2 sumsq pre-adds + residual add (lo)   ~4.1us
  DVE     psum-scale mul x2 + residual add (hi)  ~3.6us
Loads stream on the qSP HWDGE ring; stores are issued per-tile on the
qAct ring (one tile delayed so the waiting store never head-of-line
blocks the ACT compute queue) and drain concurrently with the loads.
"""

import numpy as np

import concourse.tile as tile
from concourse import bacc, mybir
from concourse.bass_utils import run_bass_kernel_spmd

C = 512  # channels
T = 8  # frames == cores
PX = 4096  # pixels per frame (64*64)
NT = 512  # pixel-tile (one PSUM bank of fp32)
NTILES = PX // NT  # 8
KC = C // 128  # 4 channel chunks

F32 = mybir.dt.float32
F32R = mybir.dt.float32r

# 1e-24/C: Rsqrt((sumsq + 1e-24)/C) reproduces the reference's
# clip(norm, 1e-12) for all non-degenerate inputs.
_EPS = 1e-24 / C

_BUILD_CACHE: dict = {}


def _build(has_bias: bool):
    """Trace + compile the per-core Tile program. Returns the Bacc."""
    nc = bacc.Bacc("TRN2", target_bir_lowering=False, debug=False, num_devices=T)

    # x and wt are declared float32r (same bits as f32): the PE rounds
    # fp32r operands internally, so typing the DMA chain fp32r satisfies
    # the BIR verifier with no extra conversion passes. Non-matmul
    # consumers read them through a .bitcast(F32) view.
    # x and out are tile-major on the host side ([tile, p, a, n]): each
    # pixel tile is one contiguous 1MB DRAM block, so a tile DMA is a
    # single contiguous 8KB line per partition.
    # wt is a-major ([a, p, j, m]) so each ci-chunk is its own contiguous
    # 2KB-per-partition block: the four chunk DMAs are queued behind x0
    # and the a-outer matmul order lets tile 0's GEMM start on chunk 0
    # while the rest are still in flight.
    x = nc.dram_tensor("x", [NTILES, 128, KC, NT], F32R, kind="ExternalInput").ap()
    wt = nc.dram_tensor("wt", [KC, 128, KC, 128], F32R, kind="ExternalInput").ap()
    out = nc.dram_tensor("out", [NTILES, 128, KC, NT], F32, kind="ExternalOutput").ap()
    beff = None
    if has_bias:
        beff = nc.dram_tensor("beff", [128, KC], F32R, kind="ExternalInput").ap()

    with tile.TileContext(nc) as tc:
        with (
            tc.tile_pool(name="const", bufs=1) as const,
            tc.tile_pool(name="xin", bufs=8) as xin,
            tc.tile_pool(name="sq", bufs=3) as sq,
            tc.tile_pool(name="red", bufs=3) as red,
            tc.tile_pool(name="sca", bufs=4) as sca,
            tc.tile_pool(name="tmp", bufs=3) as tmpp,
            tc.tile_pool(name="acc", bufs=3, space="PSUM") as accp,
            tc.tile_pool(name="stat", bufs=2, space="PSUM") as statp,
        ):
            ones_bf = const.tile([128, 128], F32)
            nc.vector.memset(ones_bf, 1.0)
            ones_b = const.tile([128, 128], F32R)
            nc.vector.tensor_copy(ones_b, ones_bf)
            eps_t = const.tile([128, 1], F32)
            nc.vector.memset(eps_t, _EPS)
            wt_sb = const.tile([128, KC, KC, 128], F32R)
            if has_bias:
                beff_sb = const.tile([128, KC], F32R)
                nc.sync.dma_start(out=beff_sb, in_=beff)

            store_q = []  # (ti, xt) pairs whose store is still to be issued

            for ti in range(NTILES):
                xt = xin.tile([128, KC, NT], F32R, tag="xt")
                nc.sync.dma_start(out=xt, in_=x[ti])
                if ti == 0:
                    # weight chunks ride the same FIFO ring right behind
                    # x0: chunk a lands just before the a-round of tile
                    # 0's GEMM needs it.
                    for a in range(KC):
                        nc.sync.dma_start(out=wt_sb[:, a, :, :], in_=wt[a])

                # per-pixel sum of squares over channels: square (ACT),
                # pairwise chunk adds (GPSIMD), then ones[128,128]
                # matmuls that reduce the partitions AND broadcast the
                # result to every output partition.
                x2 = sq.tile([128, KC, NT], F32, tag="x2", name="x2")
                nc.scalar.activation(
                    out=x2,
                    in_=xt.bitcast(F32),
                    func=mybir.ActivationFunctionType.Square,
                )
                xx = red.tile([128, 2, NT], F32R, tag="xx", name="xx")
                nc.gpsimd.tensor_add(xx[:, 0, :], x2[:, 0, :], x2[:, 1, :])
                nc.gpsimd.tensor_add(xx[:, 1, :], x2[:, 2, :], x2[:, 3, :])

                # main GEMMs. Tile 0 runs a-outer so its first rounds
                # stream against the weight chunks still in flight;
                # later tiles run j-outer so the first psum chunk (whose
                # ring buffer is reused from two tiles back) never makes
                # round a0 wait on the previous tile's combine.
                accs = []
                for jj in range(KC // 2):
                    accs.append(accp.tile([128, 2, NT], F32, tag="acc", name="acc"))
                if ti == 0:
                    order = [
                        (a, jj, q)
                        for a in range(KC)
                        for jj in range(KC // 2)
                        for q in range(2)
                    ]
                else:
                    order = [
                        (a, jj, q)
                        for jj in range(KC // 2)
                        for q in range(2)
                        for a in range(KC)
                    ]
                for a, jj, q in order:
                    nc.tensor.matmul(
                        accs[jj][:, q, :],
                        lhsT=wt_sb[:, a, jj * 2 + q, :],
                        rhs=xt[:, a, :],
                        start=(a == 0),
                        stop=(a == KC - 1),
                    )
                ssb = statp.tile([128, NT], F32, tag="stat", name="ssb")
                nc.tensor.matmul(ssb, lhsT=ones_b, rhs=xx[:, 0, :], start=True, stop=False)
                nc.tensor.matmul(ssb, lhsT=ones_b, rhs=xx[:, 1, :], start=False, stop=True)

                # s = 1/sqrt(sumsq/C + eps) = sqrt(C)/clip(norm, 1e-12).
                # (bass refuses the fused ACT Rsqrt on accuracy grounds,
                # so: ACT Sqrt then the fast DVE reciprocal.)
                stb = sca.tile([128, NT], F32R, tag="stb", name="stb")
                nc.scalar.activation(
                    out=stb,
                    in_=ssb,
                    func=mybir.ActivationFunctionType.Sqrt,
                    scale=1.0 / C,
                    bias=eps_t,
                )
                sb_s = sca.tile([128, NT], F32, tag="sb_s", name="sb_s")
                nc.vector.reciprocal_approx_fast(out=sb_s, in_=stb.bitcast(F32))

                # combine: out = x + s*acc (+beff). The psum-reading mul
                # must be DVE; the residual add is split GPSIMD/DVE so
                # neither engine exceeds the PE's ~4.2us/tile pace.
                sb_w = sb_s.unsqueeze(1).broadcast_to([128, 2, NT])
                tm = tmpp.tile([128, KC, NT], F32, tag="tm", name="tm")
                nc.vector.tensor_mul(tm[:, 0:2, :], accs[0], sb_w)
                nc.vector.tensor_mul(tm[:, 2:4, :], accs[1], sb_w)
                if has_bias:
                    for j in range(KC):
                        nc.vector.tensor_add(
                            tm[:, j, :],
                            tm[:, j, :],
                            beff_sb[:, j : j + 1].broadcast_to([128, NT]),
                        )
                nc.gpsimd.tensor_add(
                    xt[:, 0:2, :], tm[:, 0:2, :], xt[:, 0:2, :].bitcast(F32)
                )
                nc.vector.tensor_add(
                    xt[:, 2:4, :], tm[:, 2:4, :], xt[:, 2:4, :].bitcast(F32)
                )

                # stores ride the qAct HWDGE ring, one tile delayed: the
                # dma_start for tile ti-1 waits on a combine that is
                # already done by the time ACT reaches it, so the store
                # issue never stalls the square/rsqrt stream.
                store_q.append((ti, xt))
                if len(store_q) > 1:
                    si, sxt = store_q.pop(0)
                    nc.scalar.dma_start(out=out[si], in_=sxt.bitcast(F32))

            for si, sxt in store_q:
                nc.scalar.dma_start(out=out[si], in_=sxt.bitcast(F32))

    nc.compile()
    return nc


def _get_nc(has_bias: bool):
    key = has_bias
    if key not in _BUILD_CACHE:
        _BUILD_CACHE[key] = _build(has_bias)
    return _BUILD_CACHE[key]


def _prep(x, gamma, w_qkv, b_qkv, w_proj, b_proj):
    """Host-side shard + weight fold. Returns (in_maps, has_bias)."""
    x = np.asarray(x, dtype=np.float32)
    gamma = np.asarray(gamma, dtype=np.float32)
    w_qkv = np.asarray(w_qkv, dtype=np.float32)
    b_qkv = np.asarray(b_qkv, dtype=np.float32)
    w_proj = np.asarray(w_proj, dtype=np.float32)
    b_proj = np.asarray(b_proj, dtype=np.float32)

    w_v = w_qkv[2 * C : 3 * C, :]  # [cv, ci]
    b_v = b_qkv[2 * C : 3 * C]
    w_eff = (w_proj @ w_v) * gamma[None, :]  # [co, ci]
    # lhsT chunk layout [a, p, j, m]: lhsT[a*128+p -> ci, j*128+m -> co]
    wt = np.ascontiguousarray(
        w_eff.T.reshape(KC, 128, KC, 128)  # [a, p, j, m]
    )
    b_eff = (w_proj @ b_v + b_proj).astype(np.float32)
    has_bias = bool(np.any(b_eff != 0.0))

    in_maps = []
    for t in range(T):
        shard = x[0, :, t, :, :].reshape(C, PX)
        xh = np.ascontiguousarray(
            shard.reshape(KC, 128, NTILES, NT).transpose(2, 1, 0, 3)
        )
        m = {
            "x": xh,
            "wt": wt,
        }
        if has_bias:
            m["beff"] = np.ascontiguousarray(b_eff.reshape(KC, 128).T)
        in_maps.append(m)
    return in_maps, has_bias


def _run(inputs: dict, **run_kwargs):
    in_maps, has_bias = _prep(**inputs)
    nc = _get_nc(has_bias)
    res = run_bass_kernel_spmd(nc, in_maps, core_ids=list(range(T)), **run_kwargs)
    b, c, t, h, w = 1, C, T, 64, 64
    out = np.empty((b, c, t, h, w), dtype=np.float32)
    for i in range(T):
        oh = res.results[i]["out"]  # [NTILES, 128, KC, NT]
        shard = oh.transpose(2, 1, 0, 3).reshape(c, PX)
        out[0, :, i, :, :] = shard.reshape(c, h, w)
    return out, res


def kernel(**inputs) -> np.ndarray:
    out, _ = _run(inputs)
    return out


# revision 6
# speedup vs baseline: 1.3136x; 1.3136x over previous
"""Trainium2 Bass kernel for nn_AttentionBlock (B=1, C=512, T=8, H=W=64).

Math: the reference's attention has seq-len 1 (softmax over a single
element == 1.0), so o == v and Q/K never affect the output:

    out = x + s(px) * (W_eff @ x)(px) + b_eff
    W_eff = w_proj @ w_v * gamma,  w_v = w_qkv[2C:3C]
    b_eff = w_proj @ b_v + b_proj
    s(px) = sqrt(C) / clip(||x[:, px]||, 1e-12)

(The per-pixel RMS scale s commutes through the channel contraction, so
the GEMM runs on raw x and s is applied to the GEMM output.)

Sharding: data-parallel over the fused (b*t)=8 frame axis, one frame per
NeuronCore; weights replicated. Per core the frame is shipped tile-major
([tile, p, chunk, n]: channels on partitions, pixels on the free dim).

I/O rides in bf16 (x, wt, out; the host casts/upcasts): the kernel is
DMA-bound at fp32 (17.3MB/core against ~420GB/s) and the rel-l2 budget
(2e-2) dwarfs the ~3e-3 that two bf16 roundings of the identity term
cost. PSUM accumulation stays fp32.

Per-tile engine budget (8 tiles/core), balanced against durations
measured on-hardware (GPSIMD tensor ops run ~2.3x slower than DVE ones):
  PE      16 bf16 mains + 2 sumsq-ones MMs        ~4.2us
  ACT     square (bf16->f32) + sqrt               ~2.7us + store issue
  DVE     2 psum-scale muls + 3/4 residual add
          (bf16 2x mode) + reciprocal             ~4.3us
  GPSIMD  2 sumsq pre-adds + 1/4 residual add     ~4.0us
Loads stream on the qSP HWDGE ring; stores are issued per-tile on the
qAct ring (one tile delayed so a still-waiting store never head-of-line
blocks the ACT compute queue) and drain concurrently with the loads.
"""

import numpy as np

import concourse.tile as tile
from concourse import bacc, mybir
from concourse.bass_utils import run_bass_kernel_spmd

C = 512  # channels
T = 8  # frames == cores
PX = 4096  # pixels per frame (64*64)
NT = 512  # pixel-tile (one PSUM bank of fp32)
NTILES = PX // NT  # 8
KC = C // 128  # 4 channel chunks

F32 = mybir.dt.float32
F32R = mybir.dt.float32r
BF16 = mybir.dt.bfloat16

# 1e-24/C: Sqrt((sumsq + 1e-24)/C) reproduces the reference's
# clip(norm, 1e-12) for all non-degenerate inputs.
_EPS = 1e-24 / C

_BUILD_CACHE: dict = {}


def _build(has_bias: bool):
    """Trace + compile the per-core Tile program. Returns the Bacc."""
    nc = bacc.Bacc("TRN2", target_bir_lowering=False, debug=False, num_devices=T)

    # x and out are tile-major on the host side ([tile, p, a, n]): each
    # pixel tile is one contiguous 0.5MB DRAM block, a single contiguous
    # 4KB line per partition. wt is [p, a, j, m] so each partition reads
    # one contiguous 4KB row.
    x = nc.dram_tensor("x", [NTILES, 128, KC, NT], BF16, kind="ExternalInput").ap()
    wt = nc.dram_tensor("wt", [128, KC, KC, 128], BF16, kind="ExternalInput").ap()
    out = nc.dram_tensor("out", [NTILES, 128, KC, NT], BF16, kind="ExternalOutput").ap()
    beff = None
    if has_bias:
        beff = nc.dram_tensor("beff", [128, KC], BF16, kind="ExternalInput").ap()

    with tile.TileContext(nc) as tc:
        with (
            tc.tile_pool(name="const", bufs=1) as const,
            tc.tile_pool(name="xin", bufs=8) as xin,
            tc.tile_pool(name="sq", bufs=3) as sq,
            tc.tile_pool(name="red", bufs=3) as red,
            tc.tile_pool(name="sca", bufs=4) as sca,
            tc.tile_pool(name="tmp", bufs=3) as tmpp,
            tc.tile_pool(name="acc", bufs=3, space="PSUM") as accp,
            tc.tile_pool(name="stat", bufs=2, space="PSUM") as statp,
        ):
            ones_bf = const.tile([128, 128], F32)
            nc.vector.memset(ones_bf, 1.0)
            ones_b = const.tile([128, 128], F32R)
            nc.vector.tensor_copy(ones_b, ones_bf)
            eps_t = const.tile([128, 1], F32)
            nc.vector.memset(eps_t, _EPS)
            wt_sb = const.tile([128, KC, KC, 128], BF16)
            if has_bias:
                beff_sb = const.tile([128, KC], BF16)
                nc.sync.dma_start(out=beff_sb, in_=beff)

            store_q = []  # (ti, xt) pairs whose store is still to be issued

            for ti in range(NTILES):
                xt = xin.tile([128, KC, NT], BF16, tag="xt")
                nc.sync.dma_start(out=xt, in_=x[ti])
                if ti == 0:
                    # weights ride the same FIFO ring right behind x0 —
                    # x0 lands first so the ACT square chain starts ~1.4us
                    # before the first matmul needs the weights.
                    nc.sync.dma_start(out=wt_sb, in_=wt)

                # per-pixel sum of squares over channels: square (ACT),
                # pairwise chunk adds (GPSIMD), then ones[128,128]
                # matmuls that reduce the partitions AND broadcast the
                # result to every output partition.
                x2 = sq.tile([128, KC, NT], F32, tag="x2", name="x2")
                nc.scalar.activation(
                    out=x2,
                    in_=xt,
                    func=mybir.ActivationFunctionType.Square,
                )
                xx = red.tile([128, 2, NT], F32R, tag="xx", name="xx")
                nc.gpsimd.tensor_add(xx[:, 0, :], x2[:, 0, :], x2[:, 1, :])
                nc.gpsimd.tensor_add(xx[:, 1, :], x2[:, 2, :], x2[:, 3, :])

                # main GEMMs, j-outer: the first psum chunk of tile t
                # reuses the ring buffer freed by tile t-2's combine, so
                # round a0 never waits on the previous tile's scale.
                accs = []
                for jj in range(KC // 2):
                    accs.append(accp.tile([128, 2, NT], F32, tag="acc", name="acc"))
                for jj in range(KC // 2):
                    for q in range(2):
                        for a in range(KC):
                            nc.tensor.matmul(
                                accs[jj][:, q, :],
                                lhsT=wt_sb[:, a, jj * 2 + q, :],
                                rhs=xt[:, a, :],
                                start=(a == 0),
                                stop=(a == KC - 1),
                            )
                ssb = statp.tile([128, NT], F32, tag="stat", name="ssb")
                nc.tensor.matmul(ssb, lhsT=ones_b, rhs=xx[:, 0, :], start=True, stop=False)
                nc.tensor.matmul(ssb, lhsT=ones_b, rhs=xx[:, 1, :], start=False, stop=True)

                # s = 1/sqrt(sumsq/C + eps) = sqrt(C)/clip(norm, 1e-12)
                stb = sca.tile([128, NT], F32R, tag="stb", name="stb")
                nc.scalar.activation(
                    out=stb,
                    in_=ssb,
                    func=mybir.ActivationFunctionType.Sqrt,
                    scale=1.0 / C,
                    bias=eps_t,
                )
                sb_s = sca.tile([128, NT], F32, tag="sb_s", name="sb_s")
                nc.vector.reciprocal_approx_fast(out=sb_s, in_=stb.bitcast(F32))

                # combine: out = x + s*acc (+beff). The psum-reading muls
                # must be DVE; the residual add is split 3:1 DVE:GPSIMD
                # (DVE's all-bf16 add runs in 2x mode, GPSIMD's does not)
                # so neither engine exceeds the PE's ~4.2us/tile pace.
                sb_w = sb_s.unsqueeze(1).broadcast_to([128, 2, NT])
                tm = tmpp.tile([128, KC, NT], BF16, tag="tm", name="tm")
                nc.vector.tensor_mul(tm[:, 0:2, :], accs[0], sb_w)
                nc.vector.tensor_mul(tm[:, 2:4, :], accs[1], sb_w)
                if has_bias:
                    for j in range(KC):
                        nc.vector.tensor_add(
                            tm[:, j, :],
                            tm[:, j, :],
                            beff_sb[:, j : j + 1].broadcast_to([128, NT]),
                        )
                nc.vector.tensor_add(xt[:, 0:3, :], tm[:, 0:3, :], xt[:, 0:3, :])
                nc.gpsimd.tensor_add(xt[:, 3, :], tm[:, 3, :], xt[:, 3, :])

                # stores ride the qAct HWDGE ring, one tile delayed: the
                # dma_start for tile ti-1 waits on a combine that is
                # already done by the time ACT reaches it, so the store
                # issue never stalls the square/sqrt stream.
                store_q.append((ti, xt))
                if len(store_q) > 1:
                    si, sxt = store_q.pop(0)
                    nc.scalar.dma_start(out=out[si], in_=sxt)

            for si, sxt in store_q:
                nc.scalar.dma_start(out=out[si], in_=sxt)

    nc.compile()
    return nc


def _get_nc(has_bias: bool):
    key = has_bias
    if key not in _BUILD_CACHE:
        _BUILD_CACHE[key] = _build(has_bias)
    return _BUILD_CACHE[key]


def _prep(x, gamma, w_qkv, b_qkv, w_proj, b_proj):
    """Host-side shard + weight fold. Returns (in_maps, has_bias)."""
    bf16 = mybir.dt.np(BF16)
    x = np.asarray(x, dtype=np.float32)
    gamma = np.asarray(gamma, dtype=np.float32)
    w_qkv = np.asarray(w_qkv, dtype=np.float32)
    b_qkv = np.asarray(b_qkv, dtype=np.float32)
    w_proj = np.asarray(w_proj, dtype=np.float32)
    b_proj = np.asarray(b_proj, dtype=np.float32)

    w_v = w_qkv[2 * C : 3 * C, :]  # [cv, ci]
    b_v = b_qkv[2 * C : 3 * C]
    w_eff = (w_proj @ w_v) * gamma[None, :]  # [co, ci]
    # lhsT layout [p, a, j, m]: lhsT[a*128+p -> ci, j*128+m -> co]
    wt = np.ascontiguousarray(
        w_eff.T.reshape(KC, 128, KC, 128).transpose(1, 0, 2, 3).astype(bf16)
    )
    b_eff = (w_proj @ b_v + b_proj).astype(np.float32)
    has_bias = bool(np.any(b_eff != 0.0))

    xb = x.astype(bf16)
    in_maps = []
    for t in range(T):
        shard = xb[0, :, t, :, :].reshape(C, PX)
        xh = np.ascontiguousarray(
            shard.reshape(KC, 128, NTILES, NT).transpose(2, 1, 0, 3)
        )
        m = {
            "x": xh,
            "wt": wt,
        }
        if has_bias:
            m["beff"] = np.ascontiguousarray(b_eff.reshape(KC, 128).T.astype(bf16))
        in_maps.append(m)
    return in_maps, has_bias


def _run(inputs: dict, **run_kwargs):
    in_maps, has_bias = _prep(**inputs)
    nc = _get_nc(has_bias)
    res = run_bass_kernel_spmd(nc, in_maps, core_ids=list(range(T)), **run_kwargs)
    b, c, t, h, w = 1, C, T, 64, 64
    out = np.empty((b, c, t, h, w), dtype=np.float32)
    for i in range(T):
        oh = res.results[i]["out"].astype(np.float32)  # [NTILES, 128, KC, NT]
        shard = oh.transpose(2, 1, 0, 3).reshape(c, PX)
        out[0, :, i, :, :] = shard.reshape(c, h, w)
    return out, res


def kernel(**inputs) -> np.ndarray:
    out, _ = _run(inputs)
    return out
